# revision 15
# baseline (speedup 1.0000x reference)
"""Trainium2 Bass kernel for ExtractorLoss (PSD SNR loss).

loss = -mean_b( 10*log10( (mean wanted psd) / (mean unwanted psd) ) )
with psd[b,g] = (x @ cos_g)^2 + (x @ sin_g)^2 over a 201-bin frequency grid.

Math: grid frequencies are g/1800 cycles/sample (g = grid_bpm in 40..240,
fs = 30 Hz), so the DFT basis has period 1800 and quarter-period structure:
shifting tau by 450 multiplies (cos,sin) by a rotation of angle (pi/2)g.
Folding the eight 450-sample segments of x on host gives, per batch row,
four folded vectors (u0 for g%4==0, u2 for g%4==2, uo/vo for odd g) and a
GEMM with contraction 450 — 9696 PE column-cycles per core vs 12928 for the
half-period fold and 51712 for the naive GEMM.

Inputs are quantized to fp8 e4m3 on host (validated: final rel err ~6e-4
vs the 2e-2 gate); this halves HBM traffic. Per core the packed x tensor is
[450, 2650] = [u0|u2|uo|vo (4x512 batch cols) | even basis 202 | odd-u 202
... odd bases 200+200], stored k-tile-major so each of 4 DMAs lands one
128-row contraction tile with one contiguous ~2.6KB descriptor/partition.

Raw Bacc kernel, no TileContext and no Block barriers: the NEFF-level
epilogue (per-semaphore resets, ~6us on the Tensor engine) starts on each
engine as soon as its own stream ends, so engines that finish early hide
the fixed epilogue under the remaining work. All cross-engine ordering is
explicit semaphores; hardware semaphores start at 0 (walrus clears all of
S[3..255] in its own epilogue every execution, and the runtime zeroes them
at NEFF load).

Pipeline: Sync streams 4 k-tile DMAs; PE runs ~20 zero matmuls first (fed
by a GpSimd memset) purely to flip the HAM clock gate from 1.2 to 2.4 GHz
before real data lands, then accumulates 4 matmuls per (k,m) into one PSUM
bank per m-tile; Scalar fetches the mask (after the x stream clears) and
runs Square-with-accumulate per m-tile (sq -> SBUF bf16, total -> f32);
DVE does one fused tensor_tensor_reduce per m-tile (sq*mask, sum) for the
wanted-band sum; Sync DMAs the [128,8] result out. Log/mean run on host.
"""

import functools
import sys

import numpy as np
import ml_dtypes

if "/opt/trn_rl_repo" not in sys.path:
    sys.path.insert(0, "/opt/trn_rl_repo")

# Problem constants (fixed by the problem spec).
B, T, NG = 4096, 3600, 201
NCORES = 8
BS = B // NCORES          # 512 batch rows per core
MT = BS // 128            # 4 output partition tiles per core
TF = T // 8               # 450 folded contraction length
N0, N2, NO = 51, 50, 100  # grid bins with g%4==0 / g%4==2 / odd g
XCOL = 4 * BS             # 2048 x cols (u0|u2|uo|vo)
BE0 = XCOL                # even-class basis [C0|S0|C2|S2]
BOU = BE0 + 2 * (N0 + N2)        # odd basis for uo [Co|So]
BOV = BOU + 2 * NO               # odd basis for vo [-sgn*So|sgn*Co]
PCOL = BOV + 2 * NO              # 2650 packed cols per fold row
FD = 2 * (N0 + N2 + NO)          # 402 psd cols per m-tile
FDP = FD + 2                     # padded stride (4B-aligns each m slice)
KS = [128, 128, 128, TF - 384]   # contraction k-tiles (128,128,128,66)

E4M3 = ml_dtypes.float8_e4m3
BF16 = ml_dtypes.bfloat16
NWARM = 20                # big HAM warmup matmuls (~2.1us at the cold rate)
NWTAIL = 88               # small tail warmups; must outlast the first DMA


@functools.lru_cache(maxsize=1)
def _build_program():
    import concourse.bacc as bacc
    import concourse.mybir as mybir
    from contextlib import ExitStack

    f32 = mybir.dt.float32
    bf16 = mybir.dt.bfloat16
    fp8 = mybir.dt.float8e4

    nc = bacc.Bacc()
    xq01 = nc.declare_dram_parameter("xq01", [128, 2, PCOL], fp8, isOutput=False)
    xq2 = nc.declare_dram_parameter("xq2", [128, PCOL], fp8, isOutput=False)
    xr = nc.declare_dram_parameter("xr", [KS[3], PCOL], fp8, isOutput=False)
    maskd = nc.declare_dram_parameter("mask", [128, MT, FDP], bf16, isOutput=False)
    outd = nc.declare_dram_parameter("out", [128, 2 * MT], f32, isOutput=True)

    with ExitStack() as ctx:
        xsb = ctx.enter_context(nc.sbuf_tensor("xsb", [128, 4, PCOL], fp8))
        masksb = ctx.enter_context(nc.sbuf_tensor("masksb", [128, MT, FDP], bf16))
        sq = ctx.enter_context(nc.sbuf_tensor("sq", [128, MT, FDP], bf16))
        junk = ctx.enter_context(nc.sbuf_tensor("junk", [128, MT, FDP], bf16))
        outsb = ctx.enter_context(nc.sbuf_tensor("outsb", [128, 2 * MT], f32))
        warm = ctx.enter_context(nc.sbuf_tensor("warm", [128, 128], bf16))
        ps = ctx.enter_context(nc.psum_tensor("ps", [128, 8, 512], f32))

        dsems = [ctx.enter_context(nc.semaphore(f"dsem{k}")) for k in range(4)]
        msem = ctx.enter_context(nc.semaphore("msem"))
        wsem = ctx.enter_context(nc.semaphore("wsem"))
        pesem = ctx.enter_context(nc.semaphore("pesem"))
        actsem = ctx.enter_context(nc.semaphore("actsem"))
        dvesem = ctx.enter_context(nc.semaphore("dvesem"))
        osem = ctx.enter_context(nc.semaphore("osem"))

        # --- GpSimd: zero the warmup operand, then done for the run.
        nc.gpsimd.memset(warm[:], 0).then_inc(wsem, 1)

        # --- Sync: stream the x k-tiles, then write the result out.
        # k-tiles 0+1 ride one DMA (5300B/partition descriptors run ~8%
        # faster per byte and pay one completion round-trip, not two).
        nc.sync.dma_start(out=xsb[:, 0:2, :], in_=xq01[:]).then_inc(dsems[1], 16)
        nc.sync.dma_start(out=xsb[:, 2, :], in_=xq2[:]).then_inc(dsems[2], 16)
        nc.sync.dma_start(out=xsb[: KS[3], 3, :], in_=xr[:, :]).then_inc(
            dsems[3], 16
        )
        # Mask rides the same HWDGE ring: strictly after the x stream, so it
        # cannot steal SDMA packet slots from the k-tiles PE is waiting on.
        nc.sync.dma_start(out=masksb[:], in_=maskd[:]).then_inc(msem, 16)
        nc.sync.wait_ge(dvesem, 1)
        nc.sync.dma_start(out=outd[:], in_=outsb[:]).then_inc(osem, 16)

        # --- Scalar: mask DMA (after the x stream is clear of the rings),
        # then per-m Square with accumulated row totals.
        for m in range(MT):
            nc.scalar.wait_ge(pesem, m + 1)
            nc.scalar.activation(
                sq[:, m, 0:FD],
                ps[:, m, 0:FD],
                mybir.ActivationFunctionType.Square,
                accum_out=outsb[:, m : m + 1],
            ).then_inc(actsem, 1)

        # --- Tensor: HAM warmup on zeros, then the folded-DFT GEMM.
        nc.tensor.wait_ge(wsem, 1)
        for _ in range(NWARM):
            nc.tensor.matmul(
                ps[:, 4, 0:128], lhsT=warm[:], rhs=warm[:], start=True, stop=True
            )
        for _ in range(NWTAIL):
            nc.tensor.matmul(
                ps[:, 4, 0:16], lhsT=warm[:], rhs=warm[:, 0:16], start=True, stop=True
            )
        nc.tensor.wait_ge(dsems[1], 16)
        for k in range(2):
            kk = KS[k]
            for m in range(MT):
                c = m * 128
                # One accumulation group per PSUM bank: the first matmul
                # (start=True) clears the whole bank, the last (stop=True)
                # closes the group; everything between accumulates.
                nc.tensor.matmul(
                    ps[:, m, 0 : 2 * N0],
                    lhsT=xsb[:kk, k, c : c + 128],
                    rhs=xsb[:kk, k, BE0 : BE0 + 2 * N0],
                    start=(k == 0),
                    stop=False,
                )
                nc.tensor.matmul(
                    ps[:, m, 2 * N0 : 2 * (N0 + N2)],
                    lhsT=xsb[:kk, k, BS + c : BS + c + 128],
                    rhs=xsb[:kk, k, BE0 + 2 * N0 : BOU],
                    start=False,
                    stop=False,
                )
                nc.tensor.matmul(
                    ps[:, m, 2 * (N0 + N2) : FD],
                    lhsT=xsb[:kk, k, 2 * BS + c : 2 * BS + c + 128],
                    rhs=xsb[:kk, k, BOU:BOV],
                    start=False,
                    stop=False,
                )
                last = nc.tensor.matmul(
                    ps[:, m, 2 * (N0 + N2) : FD],
                    lhsT=xsb[:kk, k, 3 * BS + c : 3 * BS + c + 128],
                    rhs=xsb[:kk, k, BOV:PCOL],
                    start=False,
                    stop=(k == 3),
                )
        # Last two k-tiles per m-tile back to back so each m's PSUM bank
        # closes (and its epilogue starts) as early as possible.
        nc.tensor.wait_ge(dsems[2], 16)
        nc.tensor.wait_ge(dsems[3], 16)
        for m in range(MT):
            c = m * 128
            for k in (2, 3):
                kk = KS[k]
                nc.tensor.matmul(
                    ps[:, m, 0 : 2 * N0],
                    lhsT=xsb[:kk, k, c : c + 128],
                    rhs=xsb[:kk, k, BE0 : BE0 + 2 * N0],
                    start=False,
                    stop=False,
                )
                nc.tensor.matmul(
                    ps[:, m, 2 * N0 : 2 * (N0 + N2)],
                    lhsT=xsb[:kk, k, BS + c : BS + c + 128],
                    rhs=xsb[:kk, k, BE0 + 2 * N0 : BOU],
                    start=False,
                    stop=False,
                )
                nc.tensor.matmul(
                    ps[:, m, 2 * (N0 + N2) : FD],
                    lhsT=xsb[:kk, k, 2 * BS + c : 2 * BS + c + 128],
                    rhs=xsb[:kk, k, BOU:BOV],
                    start=False,
                    stop=False,
                )
                last = nc.tensor.matmul(
                    ps[:, m, 2 * (N0 + N2) : FD],
                    lhsT=xsb[:kk, k, 3 * BS + c : 3 * BS + c + 128],
                    rhs=xsb[:kk, k, BOV:PCOL],
                    start=False,
                    stop=(k == 3),
                )
            last.then_inc(pesem, 1)

        # --- DVE: fused (sq * mask) multiply-reduce per m for the wanted sum.
        nc.vector.wait_ge(msem, 16)
        for m in range(MT):
            nc.vector.wait_ge(actsem, m + 1)
            stt = nc.vector.scalar_tensor_tensor(
                out=junk[:, m, 0:FD],
                in0=sq[:, m, 0:FD],
                scalar=1.0,
                in1=masksb[:, m, 0:FD],
                op0=mybir.AluOpType.mult,
                op1=mybir.AluOpType.mult,
                accum_out=outsb[:, MT + m : MT + m + 1],
            )
        stt.then_inc(dvesem, 1)

    nc.finalize()
    return nc


def _host_prep(x, f_true_bpm, fs, delta_bpm, sampling_bpm, fmin_bpm, fmax_bpm):
    fs = int(fs)
    delta = int(delta_bpm)
    samp = int(sampling_bpm)
    fmin = int(fmin_bpm)
    fmax = int(fmax_bpm)

    n_grid = (fmax - fmin) // samp + 1
    assert n_grid == NG and fs == 30 and samp == 1, (n_grid, fs, samp)
    grid = fmin + samp * np.arange(n_grid, dtype=np.int64)
    g0 = grid[grid % 4 == 0]          # 51 bins
    g2 = grid[grid % 4 == 2]          # 50 bins
    go = grid[grid % 2 == 1]          # 100 bins
    assert len(g0) == N0 and len(g2) == N2 and len(go) == NO

    # Quarter-period folded basis over tau in [0, 450).
    tau = np.arange(TF, dtype=np.float64)
    th = lambda g: 2.0 * np.pi * tau[:, None] * g[None, :] / 1800.0
    C0, S0 = np.cos(th(g0)), np.sin(th(g0))
    C2, S2 = np.cos(th(g2)), np.sin(th(g2))
    Co, So = np.cos(th(go)), np.sin(th(go))
    sgn = np.where(go % 4 == 1, 1.0, -1.0)[None, :]
    basis = np.empty((TF, PCOL - XCOL), dtype=np.float64)
    basis[:, 0:N0] = C0
    basis[:, N0 : 2 * N0] = S0
    basis[:, 2 * N0 : 2 * N0 + N2] = C2
    basis[:, 2 * N0 + N2 : 2 * (N0 + N2)] = S2
    o = 2 * (N0 + N2)
    basis[:, o : o + NO] = Co
    basis[:, o + NO : o + 2 * NO] = So
    basis[:, o + 2 * NO : o + 3 * NO] = -sgn * So
    basis[:, o + 3 * NO : o + 4 * NO] = sgn * Co
    basis8 = basis.astype(E4M3)

    # Fold x: 8 segments of 450 with per-class segment coefficients.
    s = x.astype(np.float64).reshape(B, 8, TF)
    e, oo = s[:, 0::2], s[:, 1::2]     # even/odd segment groups [B,4,TF]
    u0 = (e.sum(1) + oo.sum(1)).astype(E4M3)
    u2 = (e.sum(1) - oo.sum(1)).astype(E4M3)
    alt = np.array([1.0, -1.0, 1.0, -1.0])
    uo = np.einsum("j,bjt->bt", alt, e).astype(E4M3)
    vo = np.einsum("j,bjt->bt", alt, oo).astype(E4M3)

    # Wanted-band mask in [E0|E0|E2|E2|O|O] column order, bf16.
    f64 = f_true_bpm.astype(np.int64)
    w0 = np.abs(g0[None, :] - f64[:, None]) <= delta
    w2 = np.abs(g2[None, :] - f64[:, None]) <= delta
    wo = np.abs(go[None, :] - f64[:, None]) <= delta
    pad = np.zeros((B, FDP - FD))
    mask = np.concatenate([w0, w0, w2, w2, wo, wo, pad], axis=1).astype(BF16)

    in_maps = []
    for c in range(NCORES):
        sl = slice(c * BS, (c + 1) * BS)
        xbp = np.empty((TF, PCOL), dtype=E4M3)
        xbp[:, 0:BS] = u0[sl].T
        xbp[:, BS : 2 * BS] = u2[sl].T
        xbp[:, 2 * BS : 3 * BS] = uo[sl].T
        xbp[:, 3 * BS : XCOL] = vo[sl].T
        xbp[:, XCOL:] = basis8
        # k-tile-major: partition p holds fold rows p, 128+p (+256+p).
        xq01c = np.ascontiguousarray(
            xbp[0:256].reshape(2, 128, PCOL).transpose(1, 0, 2)
        )
        xq2c = np.ascontiguousarray(xbp[256:384])
        xrc = np.ascontiguousarray(xbp[384:TF])
        mc = np.ascontiguousarray(
            mask[sl].reshape(MT, 128, FDP).transpose(1, 0, 2)
        )
        in_maps.append({"xq01": xq01c, "xq2": xq2c, "xr": xrc, "mask": mc})

    n_wanted = 2 * delta // samp + 1
    n_unwanted = n_grid - n_wanted
    return in_maps, n_wanted, n_unwanted


def _finalize(outs, n_wanted, n_unwanted):
    # outs: per core [128, 8] f32 = [total m0..m3 | wanted m0..m3] per row.
    snrs = []
    for o in outs:
        o = np.asarray(o, dtype=np.float64)
        total = o[:, 0:MT].T.reshape(-1)    # batch row m*128+p
        wanted = o[:, MT : 2 * MT].T.reshape(-1)
        term1 = wanted / n_wanted
        term2 = (total - wanted) / n_unwanted
        snrs.append(10.0 * np.log10(term1 / term2))
    return np.array(-np.concatenate(snrs).mean(), dtype=np.float32)


def kernel(x, f_true_bpm, fs, delta_bpm, sampling_bpm, fmin_bpm, fmax_bpm):
    from concourse.bass_utils import run_bass_kernel_spmd

    x = np.asarray(x, dtype=np.float32)
    f_true_bpm = np.asarray(f_true_bpm)
    in_maps, n_wanted, n_unwanted = _host_prep(
        x, f_true_bpm, fs, delta_bpm, sampling_bpm, fmin_bpm, fmax_bpm
    )
    nc = _build_program()
    res = run_bass_kernel_spmd(nc, in_maps, core_ids=list(range(NCORES)))
    outs = [r["out"] for r in res.results]
    return _finalize(outs, n_wanted, n_unwanted)


# revision 16
# speedup vs baseline: 1.0991x; 1.0991x over previous
"""Trainium2 Bass kernel for ExtractorLoss (PSD SNR loss).

loss = -mean_b( 10*log10( (mean wanted psd) / (mean unwanted psd) ) )
with psd[b,g] = (x @ cos_g)^2 + (x @ sin_g)^2 over a 201-bin frequency grid.

Math: grid frequencies are g/1800 cycles/sample (g = grid_bpm in 40..240,
fs = 30 Hz), so the DFT basis has period 1800 and quarter-period structure:
shifting tau by 450 multiplies (cos,sin) by a rotation of angle (pi/2)g.
Folding the eight 450-sample segments of x on host gives, per batch row,
four folded vectors (u0 for g%4==0, u2 for g%4==2, uo/vo for odd g) and a
GEMM with contraction 450 — 9696 PE column-cycles per core vs 12928 for the
half-period fold and 51712 for the naive GEMM.

Inputs are quantized to fp8 e4m3 on host (validated: final rel err ~6e-4
vs the 2e-2 gate); this halves HBM traffic. Per core the packed x tensor is
[450, 2650] = [u0|u2|uo|vo (4x512 batch cols) | even basis 202 | odd-u 202
... odd bases 200+200], stored k-tile-major so each of 4 DMAs lands one
128-row contraction tile with one contiguous ~2.6KB descriptor/partition.

Raw Bacc kernel, no TileContext and no Block barriers: the NEFF-level
epilogue (per-semaphore resets, ~6us on the Tensor engine) starts on each
engine as soon as its own stream ends, so engines that finish early hide
the fixed epilogue under the remaining work. All cross-engine ordering is
explicit semaphores; hardware semaphores start at 0 (walrus clears all of
S[3..255] in its own epilogue every execution, and the runtime zeroes them
at NEFF load).

Pipeline: Sync streams 4 k-tile DMAs; PE runs ~20 zero matmuls first (fed
by a GpSimd memset) purely to flip the HAM clock gate from 1.2 to 2.4 GHz
before real data lands, then accumulates 4 matmuls per (k,m) into one PSUM
bank per m-tile; Scalar fetches the mask (after the x stream clears) and
runs Square-with-accumulate per m-tile (sq -> SBUF bf16, total -> f32);
DVE does one fused tensor_tensor_reduce per m-tile (sq*mask, sum) for the
wanted-band sum; Sync DMAs the [128,8] result out. Log/mean run on host.
"""

import functools
import sys

import numpy as np
import ml_dtypes

if "/opt/trn_rl_repo" not in sys.path:
    sys.path.insert(0, "/opt/trn_rl_repo")

# Problem constants (fixed by the problem spec).
B, T, NG = 4096, 3600, 201
NCORES = 8
BS = B // NCORES          # 512 batch rows per core
MT = BS // 128            # 4 output partition tiles per core
TF = T // 8               # 450 folded contraction length
N0, N2, NO = 51, 50, 100  # grid bins with g%4==0 / g%4==2 / odd g
XCOL = 4 * BS             # 2048 x cols (u0|u2|uo|vo)
BE0 = XCOL                # even-class basis [C0|S0|C2|S2]
BOU = BE0 + 2 * (N0 + N2)        # odd basis for uo [Co|So]
BOV = BOU + 2 * NO               # odd basis for vo [-sgn*So|sgn*Co]
PCOL = BOV + 2 * NO              # 2650 packed cols per fold row
FD = 2 * (N0 + N2 + NO)          # 402 psd cols per m-tile
FDP = FD + 2                     # padded stride (4B-aligns each m slice)
KS = [128, 128, 128, TF - 384]   # contraction k-tiles (128,128,128,66)

E4M3 = ml_dtypes.float8_e4m3
BF16 = ml_dtypes.bfloat16
NWARM = 20                # big HAM warmup matmuls (~2.1us at the cold rate)
NWTAIL = 30               # small tail warmups bridging to the first k-tile


@functools.lru_cache(maxsize=1)
def _build_program():
    import concourse.bacc as bacc
    import concourse.mybir as mybir
    from contextlib import ExitStack

    f32 = mybir.dt.float32
    bf16 = mybir.dt.bfloat16
    fp8 = mybir.dt.float8e4

    nc = bacc.Bacc()
    xq = nc.declare_dram_parameter("xq", [128, 3, PCOL], fp8, isOutput=False)
    xr = nc.declare_dram_parameter("xr", [KS[3], PCOL], fp8, isOutput=False)
    maskd = nc.declare_dram_parameter("mask", [128, MT, FDP], bf16, isOutput=False)
    outd = nc.declare_dram_parameter("out", [128, 2 * MT], f32, isOutput=True)

    with ExitStack() as ctx:
        xsb = ctx.enter_context(nc.sbuf_tensor("xsb", [128, 4, PCOL], fp8))
        masksb = ctx.enter_context(nc.sbuf_tensor("masksb", [128, MT, FDP], bf16))
        sq = ctx.enter_context(nc.sbuf_tensor("sq", [128, MT, FDP], bf16))
        junk = ctx.enter_context(nc.sbuf_tensor("junk", [128, MT, FDP], bf16))
        outsb = ctx.enter_context(nc.sbuf_tensor("outsb", [128, 2 * MT], f32))
        warm = ctx.enter_context(nc.sbuf_tensor("warm", [128, 128], bf16))
        ps = ctx.enter_context(nc.psum_tensor("ps", [128, 8, 512], f32))

        dsems = [ctx.enter_context(nc.semaphore(f"dsem{k}")) for k in range(4)]
        msem = ctx.enter_context(nc.semaphore("msem"))
        wsem = ctx.enter_context(nc.semaphore("wsem"))
        pesem = ctx.enter_context(nc.semaphore("pesem"))
        actsem = ctx.enter_context(nc.semaphore("actsem"))
        dvesem = ctx.enter_context(nc.semaphore("dvesem"))
        osem = ctx.enter_context(nc.semaphore("osem"))

        # --- GpSimd: zero the warmup operand, then done for the run.
        nc.gpsimd.memset(warm[:], 0).then_inc(wsem, 1)

        # --- Sync: stream the x k-tiles, then write the result out.
        for k in range(3):
            nc.sync.dma_start(out=xsb[:, k, :], in_=xq[:, k, :]).then_inc(
                dsems[k], 16
            )
        nc.sync.dma_start(out=xsb[: KS[3], 3, :], in_=xr[:, :]).then_inc(
            dsems[3], 16
        )
        # Mask rides the same HWDGE ring: strictly after the x stream, so it
        # cannot steal SDMA packet slots from the k-tiles PE is waiting on.
        nc.sync.dma_start(out=masksb[:], in_=maskd[:]).then_inc(msem, 16)
        nc.sync.wait_ge(dvesem, 1)
        nc.sync.dma_start(out=outd[:], in_=outsb[:]).then_inc(osem, 16)

        # --- Scalar: mask DMA (after the x stream is clear of the rings),
        # then per-m Square with accumulated row totals.
        for m in range(MT):
            nc.scalar.wait_ge(pesem, m + 1)
            nc.scalar.activation(
                sq[:, m, 0:FD],
                ps[:, m, 0:FD],
                mybir.ActivationFunctionType.Square,
                accum_out=outsb[:, m : m + 1],
            ).then_inc(actsem, 1)

        # --- Tensor: HAM warmup on zeros, then the folded-DFT GEMM.
        nc.tensor.wait_ge(wsem, 1)
        for _ in range(NWARM):
            nc.tensor.matmul(
                ps[:, 4, 0:128], lhsT=warm[:], rhs=warm[:], start=True, stop=True
            )
        for _ in range(NWTAIL):
            nc.tensor.matmul(
                ps[:, 4, 0:16], lhsT=warm[:], rhs=warm[:, 0:16], start=True, stop=True
            )
        for k in range(2):
            kk = KS[k]
            nc.tensor.wait_ge(dsems[k], 16)
            for m in range(MT):
                c = m * 128
                # One accumulation group per PSUM bank: the first matmul
                # (start=True) clears the whole bank, the last (stop=True)
                # closes the group; everything between accumulates.
                nc.tensor.matmul(
                    ps[:, m, 0 : 2 * N0],
                    lhsT=xsb[:kk, k, c : c + 128],
                    rhs=xsb[:kk, k, BE0 : BE0 + 2 * N0],
                    start=(k == 0),
                    stop=False,
                )
                nc.tensor.matmul(
                    ps[:, m, 2 * N0 : 2 * (N0 + N2)],
                    lhsT=xsb[:kk, k, BS + c : BS + c + 128],
                    rhs=xsb[:kk, k, BE0 + 2 * N0 : BOU],
                    start=False,
                    stop=False,
                )
                nc.tensor.matmul(
                    ps[:, m, 2 * (N0 + N2) : FD],
                    lhsT=xsb[:kk, k, 2 * BS + c : 2 * BS + c + 128],
                    rhs=xsb[:kk, k, BOU:BOV],
                    start=False,
                    stop=False,
                )
                last = nc.tensor.matmul(
                    ps[:, m, 2 * (N0 + N2) : FD],
                    lhsT=xsb[:kk, k, 3 * BS + c : 3 * BS + c + 128],
                    rhs=xsb[:kk, k, BOV:PCOL],
                    start=False,
                    stop=(k == 3),
                )
        # Last two k-tiles per m-tile back to back so each m's PSUM bank
        # closes (and its epilogue starts) as early as possible.
        nc.tensor.wait_ge(dsems[2], 16)
        nc.tensor.wait_ge(dsems[3], 16)
        for m in range(MT):
            c = m * 128
            for k in (2, 3):
                kk = KS[k]
                nc.tensor.matmul(
                    ps[:, m, 0 : 2 * N0],
                    lhsT=xsb[:kk, k, c : c + 128],
                    rhs=xsb[:kk, k, BE0 : BE0 + 2 * N0],
                    start=False,
                    stop=False,
                )
                nc.tensor.matmul(
                    ps[:, m, 2 * N0 : 2 * (N0 + N2)],
                    lhsT=xsb[:kk, k, BS + c : BS + c + 128],
                    rhs=xsb[:kk, k, BE0 + 2 * N0 : BOU],
                    start=False,
                    stop=False,
                )
                nc.tensor.matmul(
                    ps[:, m, 2 * (N0 + N2) : FD],
                    lhsT=xsb[:kk, k, 2 * BS + c : 2 * BS + c + 128],
                    rhs=xsb[:kk, k, BOU:BOV],
                    start=False,
                    stop=False,
                )
                last = nc.tensor.matmul(
                    ps[:, m, 2 * (N0 + N2) : FD],
                    lhsT=xsb[:kk, k, 3 * BS + c : 3 * BS + c + 128],
                    rhs=xsb[:kk, k, BOV:PCOL],
                    start=False,
                    stop=(k == 3),
                )
            last.then_inc(pesem, 1)

        # --- DVE: fused (sq * mask) multiply-reduce per m for the wanted sum.
        nc.vector.wait_ge(msem, 16)
        for m in range(MT):
            nc.vector.wait_ge(actsem, m + 1)
            stt = nc.vector.scalar_tensor_tensor(
                out=junk[:, m, 0:FD],
                in0=sq[:, m, 0:FD],
                scalar=1.0,
                in1=masksb[:, m, 0:FD],
                op0=mybir.AluOpType.mult,
                op1=mybir.AluOpType.mult,
                accum_out=outsb[:, MT + m : MT + m + 1],
            )
        stt.then_inc(dvesem, 1)

    nc.finalize()
    return nc


def _host_prep(x, f_true_bpm, fs, delta_bpm, sampling_bpm, fmin_bpm, fmax_bpm):
    fs = int(fs)
    delta = int(delta_bpm)
    samp = int(sampling_bpm)
    fmin = int(fmin_bpm)
    fmax = int(fmax_bpm)

    n_grid = (fmax - fmin) // samp + 1
    assert n_grid == NG and fs == 30 and samp == 1, (n_grid, fs, samp)
    grid = fmin + samp * np.arange(n_grid, dtype=np.int64)
    g0 = grid[grid % 4 == 0]          # 51 bins
    g2 = grid[grid % 4 == 2]          # 50 bins
    go = grid[grid % 2 == 1]          # 100 bins
    assert len(g0) == N0 and len(g2) == N2 and len(go) == NO

    # Quarter-period folded basis over tau in [0, 450).
    tau = np.arange(TF, dtype=np.float64)
    th = lambda g: 2.0 * np.pi * tau[:, None] * g[None, :] / 1800.0
    C0, S0 = np.cos(th(g0)), np.sin(th(g0))
    C2, S2 = np.cos(th(g2)), np.sin(th(g2))
    Co, So = np.cos(th(go)), np.sin(th(go))
    sgn = np.where(go % 4 == 1, 1.0, -1.0)[None, :]
    basis = np.empty((TF, PCOL - XCOL), dtype=np.float64)
    basis[:, 0:N0] = C0
    basis[:, N0 : 2 * N0] = S0
    basis[:, 2 * N0 : 2 * N0 + N2] = C2
    basis[:, 2 * N0 + N2 : 2 * (N0 + N2)] = S2
    o = 2 * (N0 + N2)
    basis[:, o : o + NO] = Co
    basis[:, o + NO : o + 2 * NO] = So
    basis[:, o + 2 * NO : o + 3 * NO] = -sgn * So
    basis[:, o + 3 * NO : o + 4 * NO] = sgn * Co
    basis8 = basis.astype(E4M3)

    # Fold x: 8 segments of 450 with per-class segment coefficients.
    s = x.astype(np.float64).reshape(B, 8, TF)
    e, oo = s[:, 0::2], s[:, 1::2]     # even/odd segment groups [B,4,TF]
    u0 = (e.sum(1) + oo.sum(1)).astype(E4M3)
    u2 = (e.sum(1) - oo.sum(1)).astype(E4M3)
    alt = np.array([1.0, -1.0, 1.0, -1.0])
    uo = np.einsum("j,bjt->bt", alt, e).astype(E4M3)
    vo = np.einsum("j,bjt->bt", alt, oo).astype(E4M3)

    # Wanted-band mask in [E0|E0|E2|E2|O|O] column order, bf16.
    f64 = f_true_bpm.astype(np.int64)
    w0 = np.abs(g0[None, :] - f64[:, None]) <= delta
    w2 = np.abs(g2[None, :] - f64[:, None]) <= delta
    wo = np.abs(go[None, :] - f64[:, None]) <= delta
    pad = np.zeros((B, FDP - FD))
    mask = np.concatenate([w0, w0, w2, w2, wo, wo, pad], axis=1).astype(BF16)

    in_maps = []
    for c in range(NCORES):
        sl = slice(c * BS, (c + 1) * BS)
        xbp = np.empty((TF, PCOL), dtype=E4M3)
        xbp[:, 0:BS] = u0[sl].T
        xbp[:, BS : 2 * BS] = u2[sl].T
        xbp[:, 2 * BS : 3 * BS] = uo[sl].T
        xbp[:, 3 * BS : XCOL] = vo[sl].T
        xbp[:, XCOL:] = basis8
        # k-tile-major: partition p holds fold rows p, 128+p, 256+p.
        xqc = np.ascontiguousarray(
            xbp[0:384].reshape(3, 128, PCOL).transpose(1, 0, 2)
        )
        xrc = np.ascontiguousarray(xbp[384:TF])
        mc = np.ascontiguousarray(
            mask[sl].reshape(MT, 128, FDP).transpose(1, 0, 2)
        )
        in_maps.append({"xq": xqc, "xr": xrc, "mask": mc})

    n_wanted = 2 * delta // samp + 1
    n_unwanted = n_grid - n_wanted
    return in_maps, n_wanted, n_unwanted


def _finalize(outs, n_wanted, n_unwanted):
    # outs: per core [128, 8] f32 = [total m0..m3 | wanted m0..m3] per row.
    snrs = []
    for o in outs:
        o = np.asarray(o, dtype=np.float64)
        total = o[:, 0:MT].T.reshape(-1)    # batch row m*128+p
        wanted = o[:, MT : 2 * MT].T.reshape(-1)
        term1 = wanted / n_wanted
        term2 = (total - wanted) / n_unwanted
        snrs.append(10.0 * np.log10(term1 / term2))
    return np.array(-np.concatenate(snrs).mean(), dtype=np.float32)


def kernel(x, f_true_bpm, fs, delta_bpm, sampling_bpm, fmin_bpm, fmax_bpm):
    from concourse.bass_utils import run_bass_kernel_spmd

    x = np.asarray(x, dtype=np.float32)
    f_true_bpm = np.asarray(f_true_bpm)
    in_maps, n_wanted, n_unwanted = _host_prep(
        x, f_true_bpm, fs, delta_bpm, sampling_bpm, fmin_bpm, fmax_bpm
    )
    nc = _build_program()
    res = run_bass_kernel_spmd(nc, in_maps, core_ids=list(range(NCORES)))
    outs = [r["out"] for r in res.results]
    return _finalize(outs, n_wanted, n_unwanted)
